# revision 13
# baseline (speedup 1.0000x reference)
"""CoAtNet transformer block on 8 trn2 NeuronCores, data-parallel over batch.

Layout strategy (per core, 2 batch elements):
  - Activations live "transposed": [C on partitions, (b, n) on free], which is
    exactly the DRAM layout of x (b, C, H, W).
  - LayerNorm stats via ones-matmul column sums (contraction over partitions).
  - Attention per (batch, head-group of 4, q-half of 392):
      scores^T [k-tokens part, q free] via 4-way row-packed K=32 matmuls,
      exp on ACT straight out of PSUM, multiplicative Toeplitz bias exp(B)
      applied on DVE from a host-built per-head master strip,
      A@V + denominator via 4-way col-packed matmuls (denominator rows are
      broadcast for free by an M=32 ones lhsT).
  - rel_idx is provably k - q + 812 (Toeplitz), so the (16,784,784) bias gather
    reduces to per-head [128,1552] strips built on the host.
  - All matmuls bf16 (fp32 PSUM accumulate); residuals in bf16; output fp32.
"""

import os
import sys

import numpy as np
import ml_dtypes

sys.path.insert(0, "/opt/trn_rl_repo")

H, W, C, HEADS = 28, 28, 512, 16
N = H * W            # 784
FF = 4 * C           # 2048
DH = C // HEADS      # 32
B = 16
NCORES = 8
BPC = B // NCORES    # 2 batch elements per core
P = 128
NMT = C // P         # 4 M-tiles of channels
NKT = 7              # token tiles (6x128 + 16)
NFT = FF // P        # 16
QH = N // 2          # 392 q-half
QQ = N // 4          # 196 q-quarter (attention PSUM granularity)
MCOLS = 1552         # master strip columns
EPS = 1e-5

bf16 = ml_dtypes.bfloat16


def _tok(kt):
    return P if kt < NKT - 1 else N - (NKT - 1) * P  # 128 or 16


def _build_master(rel_bias: np.ndarray) -> np.ndarray:
    """exp of the Toeplitz bias as per-head master strips.

    biasT[k, q] = rel_bias[h, k - q + 812] for k-tile t, row p (k = 128t + p):
    master[h, p, c] with c = q + 768 - 128 t, i.e. master[h,p,c] =
    exp(rel_bias[h, p - c + 1580]) (out-of-range -> exp(0)=1, only reachable
    from invalid k rows which are never contracted).
    """
    padded = np.zeros((HEADS, 1708), np.float32)
    padded[:, : rel_bias.shape[1]] = rel_bias
    e = np.exp(padded)
    idx = 1580 + np.arange(P)[:, None] - np.arange(MCOLS)[None, :]  # (128,1552)
    return np.ascontiguousarray(e[:, idx]).astype(bf16)  # (16,128,1552)


def _build_nc():
    import concourse.bass as bass  # noqa: F401
    import concourse.mybir as mybir
    import concourse.tile as tile
    from concourse import bacc
    from concourse.masks import make_identity

    fp32 = mybir.dt.float32
    bfl = mybir.dt.bfloat16
    ALU = mybir.AluOpType
    AF = mybir.ActivationFunctionType

    nc = bacc.Bacc("TRN2", target_bir_lowering=False, debug=False)

    xin = nc.dram_tensor("xin", (BPC, C, N), bfl, kind="ExternalInput").ap()
    wq = nc.dram_tensor("wq", (C, C), bfl, kind="ExternalInput").ap()
    wk = nc.dram_tensor("wk", (C, C), bfl, kind="ExternalInput").ap()
    wv = nc.dram_tensor("wv", (C, C), bfl, kind="ExternalInput").ap()
    wo = nc.dram_tensor("wo", (C, C), bfl, kind="ExternalInput").ap()
    bq = nc.dram_tensor("bq", (C,), fp32, kind="ExternalInput").ap()
    bk = nc.dram_tensor("bk", (C,), fp32, kind="ExternalInput").ap()
    bo2 = nc.dram_tensor("bo2", (C,), fp32, kind="ExternalInput").ap()
    w1 = nc.dram_tensor("w1", (C, FF), bfl, kind="ExternalInput").ap()
    b1 = nc.dram_tensor("b1", (FF,), fp32, kind="ExternalInput").ap()
    w2 = nc.dram_tensor("w2", (FF, C), bfl, kind="ExternalInput").ap()
    b2 = nc.dram_tensor("b2", (C,), fp32, kind="ExternalInput").ap()
    expe = nc.dram_tensor("expe", (HEADS, P, MCOLS), bfl, kind="ExternalInput").ap()
    out = nc.dram_tensor("out", (BPC, C, N), fp32, kind="ExternalOutput").ap()

    x_t = xin.rearrange("b (mt p) n -> mt p b n", p=P)
    out_t = out.rearrange("b (mt p) n -> mt p b n", p=P)

    with tile.TileContext(nc) as tc:
        # ---- persistent pools -------------------------------------------
        const = tc.alloc_tile_pool(name="const", bufs=1)
        act = tc.alloc_tile_pool(name="act", bufs=1)

        wqS = const.tile([P, NMT, C], bfl, tag="wqS", name="wqS")
        wkS = const.tile([P, NMT, C], bfl, tag="wkS", name="wkS")
        wvS = const.tile([P, NMT, C], bfl, tag="wvS", name="wvS")
        woS = const.tile([P, NMT, C], bfl, tag="woS", name="woS")
        for w_d, w_s in ((wq, wqS), (wk, wkS), (wv, wvS), (wo, woS)):
            nc.sync.dma_start(w_s[:], w_d.rearrange("(ks p) m -> p ks m", p=P))
        bqS = const.tile([P, NMT], fp32, tag="bqS", name="bqS")
        bkS = const.tile([P, NMT], fp32, tag="bkS", name="bkS")
        bo2S = const.tile([P, NMT], fp32, tag="bo2S", name="bo2S")
        b2S = const.tile([P, NMT], fp32, tag="b2S", name="b2S")
        for b_d, b_s in ((bq, bqS), (bk, bkS), (bo2, bo2S), (b2, b2S)):
            nc.sync.dma_start(b_s[:], b_d.rearrange("(mt p) -> p mt", p=P))
        b1S = const.tile([P, NFT], fp32, tag="b1S", name="b1S")
        nc.sync.dma_start(b1S[:], b1.rearrange("(mt p) -> p mt", p=P))

        ones_bf = const.tile([P, DH], bfl, tag="ones_bf", name="ones_bf")
        nc.any.memset(ones_bf[:], 1.0)
        ident = const.tile([P, P], bfl, tag="ident", name="ident")
        make_identity(nc, ident)

        xT = [act.tile([P, BPC, N], bfl, tag=f"xT{m}", name=f"xT{m}") for m in range(NMT)]
        for m in range(NMT):
            nc.sync.dma_start(xT[m][:], x_t[m])


        # ---- LayerNorm stats --------------------------------------------
        with tc.tile_pool(name="lnp", bufs=1) as lnp, \
             tc.tile_pool(name="lnps", bufs=2, space="PSUM") as lnps:
            xsq = [lnp.tile([P, BPC, N], bfl, tag=f"xsq{m}", name=f"xsq{m}") for m in range(NMT)]
            for m in range(NMT):
                nc.scalar.square(xsq[m][:], xT[m][:])
            must = lnp.tile([1, BPC, N], fp32, tag="must", name="must")
            sqst = lnp.tile([1, BPC, N], fp32, tag="sqst", name="sqst")
            for ch in range(4):
                b_i, h_i = ch // 2, ch % 2
                sl = (slice(None), b_i, slice(h_i * QH, (h_i + 1) * QH))
                sp = lnps.tile([P, 512], fp32)
                for ks in range(NMT):
                    nc.tensor.matmul(sp[0:1, :QH], ones_bf[:, 0:1], xT[ks][sl],
                                     start=(ks == 0), stop=(ks == NMT - 1),
                                     tile_position=(0, 0))
                    nc.tensor.matmul(sp[32:33, :QH], ones_bf[:, 0:1], xsq[ks][sl],
                                     start=(ks == 0), stop=(ks == NMT - 1),
                                     tile_position=(0, 32))
                # scale by 1/C on eviction
                nc.vector.tensor_scalar_mul(must[0:1, b_i, sl[2]], sp[0:1, :QH], 1.0 / C)
                nc.vector.tensor_scalar_mul(sqst[0:1, b_i, sl[2]], sp[32:33, :QH], 1.0 / C)
            mu = must[:]        # [1, BPC, N]
            msq = sqst[:]
            var = lnp.tile([1, BPC, N], fp32, tag="var", name="var")
            tmp1 = lnp.tile([1, BPC, N], fp32, tag="tmp1", name="tmp1")
            nc.vector.tensor_mul(tmp1[:], mu, mu)
            # var = (msq + eps) - mu^2
            nc.vector.scalar_tensor_tensor(var[:], msq, float(EPS), tmp1[:],
                                           ALU.add, ALU.subtract)
            sd = lnp.tile([1, BPC, N], fp32, tag="sd", name="sd")
            nc.scalar.activation(sd[:], var[:], AF.Sqrt)
            rsig = lnp.tile([1, BPC, N], fp32, tag="rsig", name="rsig")
            nc.vector.reciprocal_approx_accurate(rsig[:], sd[:], tmp1[:])
            negmur = lnp.tile([1, BPC, N], fp32, tag="negmur", name="negmur")
            nc.vector.scalar_tensor_tensor(negmur[:], mu, -1.0, rsig[:],
                                           ALU.mult, ALU.mult)
            rsig_bf = lnp.tile([1, BPC, N], bfl, tag="rsig_bf", name="rsig_bf")
            negmur_bf = lnp.tile([1, BPC, N], bfl, tag="negmur_bf", name="negmur_bf")
            nc.vector.tensor_copy(rsig_bf[:], rsig[:])
            nc.vector.tensor_copy(negmur_bf[:], negmur[:])
            rsigB = act.tile([P, BPC, N], bfl, tag="rsigB", name="rsigB")
            negmurB = act.tile([P, BPC, N], bfl, tag="negmurB", name="negmurB")
            nc.gpsimd.partition_broadcast(rsigB[:], rsig_bf[:])
            nc.gpsimd.partition_broadcast(negmurB[:], negmur_bf[:])

            # xn = x * rsig + (-mu * rsig)   (ln_w/ln_b folded into weights)
            xnT = [act.tile([P, BPC, N], bfl, tag=f"xnT{m}", name=f"xnT{m}") for m in range(NMT)]
            for m in range(NMT):
                nc.vector.tensor_mul(xsq[m][:], xT[m][:], rsigB[:])
                nc.vector.tensor_add(xnT[m][:], xsq[m][:], negmurB[:])

        # ---- QKV projections --------------------------------------------
        qT = [act.tile([P, BPC, N], bfl, tag=f"qT{m}", name=f"qT{m}") for m in range(NMT)]
        kT = [act.tile([P, BPC, N], bfl, tag=f"kT{m}", name=f"kT{m}") for m in range(NMT)]
        vS = [act.tile([P, NKT, C], bfl, tag=f"vS{b}", name=f"vS{b}") for b in range(BPC)]
        with tc.tile_pool(name="qkvps", bufs=3, space="PSUM") as qkvps:
            for wS, bS, dstT in ((wqS, bqS, qT), (wkS, bkS, kT)):
                for m in range(NMT):
                    for ch in range(4):
                        b_i, h_i = ch // 2, ch % 2
                        sl = (slice(None), b_i, slice(h_i * QH, (h_i + 1) * QH))
                        ps = qkvps.tile([P, 512], fp32, tag="qkv_ps", name="qkv_ps")[:, :QH]
                        for ks in range(NMT):
                            nc.tensor.matmul(ps[:], wS[:, ks, m * P:(m + 1) * P],
                                             xnT[ks][sl],
                                             start=(ks == 0), stop=(ks == NMT - 1))
                        nc.vector.tensor_scalar_add(dstT[m][sl], ps[:], bS[:, m:m + 1])
            # V in token-partition layout: v[b][tok, kt, c_out]
            for b_i in range(BPC):
                for kt in range(NKT):
                    tok = _tok(kt)
                    ps = qkvps.tile([P, C], fp32, tag="v_ps", name="v_ps")
                    for ks in range(NMT):
                        nc.tensor.matmul(
                            ps[:tok, :],
                            xnT[ks][:, b_i, kt * P:kt * P + tok],
                            wvS[:, ks, :],
                            start=(ks == 0), stop=(ks == NMT - 1))
                    nc.vector.tensor_copy(vS[b_i][:tok, kt, :], ps[:tok, :])

        # ---- attention ---------------------------------------------------
        # Scores PSUM is split into two head-pair tiles of [P, 2, 512]
        # (2 banks each, heads bank-aligned). With scps bufs=2 the next
        # iteration's first head-pair scores overlap the current exp,
        # pipelining Tensor against ACT. PSUM: 2x2 + av 2 + dn 2 = 8 banks.
        catT = [act.tile([P, BPC, N], bfl, tag=f"catT{m}", name=f"catT{m}") for m in range(NMT)]
        with tc.tile_pool(name="expp", bufs=2) as expp, \
             tc.tile_pool(name="pp", bufs=4) as pp, \
             tc.tile_pool(name="rp", bufs=2) as rp, \
             tc.tile_pool(name="scps", bufs=2, space="PSUM") as scps, \
             tc.tile_pool(name="avps", bufs=2, space="PSUM") as avps, \
             tc.tile_pool(name="dnps", bufs=2, space="PSUM") as dnps:
            for g in range(4):  # head group: heads 4g..4g+3
                eG = expp.tile([P, 4, MCOLS], bfl, tag="eG", name="eG")
                nc.sync.dma_start(eG[:], expe[4 * g:4 * g + 4].rearrange("h p c -> p h c"))
                for b_i in range(BPC):
                    for qh in range(2):
                        qsl = slice(qh * QH, (qh + 1) * QH)
                        av = avps.tile([P, 512], fp32, tag="av", name="av")[:, :QH]
                        dn = dnps.tile([P, 512], fp32, tag="dn", name="dn")[:, :QH]
                        for kt in range(NKT):
                            tok = _tok(kt)
                            ksl = slice(kt * P, kt * P + tok)
                            scH = [scps.tile([P, 2, 512], fp32, tag="sc", name="sc")
                                   for _ in range(2)]
                            c0 = 768 - P * kt + qh * QH
                            for j in range(4):
                                nc.tensor.matmul(
                                    scH[j // 2][:tok, j % 2, :QH],
                                    kT[g][32 * j:32 * j + 32, b_i, ksl],
                                    qT[g][32 * j:32 * j + 32, b_i, qsl],
                                    start=True, stop=True,
                                    tile_position=(32 * j, 0))
                            for hp in range(2):
                                pS = pp.tile([P, 2, QH], bfl, tag="pS", name="pS")
                                nc.scalar.activation(pS[:tok], scH[hp][:tok, :, :QH],
                                                     AF.Exp)
                                nc.vector.tensor_mul(
                                    pS[:tok], pS[:tok],
                                    eG[:tok, 2 * hp:2 * hp + 2, c0:c0 + QH])
                                for jj in range(2):
                                    j = 2 * hp + jj
                                    nc.tensor.matmul(
                                        av[32 * j:32 * j + 32, :],
                                        vS[b_i][:tok, kt, 128 * g + 32 * j:128 * g + 32 * j + 32],
                                        pS[:tok, jj, :],
                                        start=(kt == 0), stop=(kt == NKT - 1),
                                        tile_position=(0, 32 * j),
                                        skip_group_check=True)
                                    nc.tensor.matmul(
                                        dn[32 * j:32 * j + 32, :],
                                        ones_bf[:tok, :],
                                        pS[:tok, jj, :],
                                        start=(kt == 0), stop=(kt == NKT - 1),
                                        tile_position=(0, 32 * j),
                                        skip_group_check=True)
                        recipB = rp.tile([P, QH], fp32, tag="recipB", name="recipB")
                        scr = rp.tile([P, QH], fp32, tag="scr", name="scr")
                        nc.vector.reciprocal_approx_accurate(recipB[:], dn[:], scr[:])
                        nc.vector.tensor_mul(catT[g][:, b_i, qsl], av[:], recipB[:])

        # ---- out proj + residual ----------------------------------------
        # FFN weight DMAs issued here so they overlap the out-projection.
        ffp = tc.alloc_tile_pool(name="ffp", bufs=1)
        w1S = ffp.tile([P, NMT, FF], bfl, tag="w1S", name="w1S")
        nc.sync.dma_start(w1S[:], w1.rearrange("(ks p) m -> p ks m", p=P))
        w2S = ffp.tile([P, NFT, C], bfl, tag="w2S", name="w2S")
        nc.sync.dma_start(w2S[:], w2.rearrange("(ks p) m -> p ks m", p=P))

        x1T = [act.tile([P, BPC, N], bfl, tag=f"x1T{m}", name=f"x1T{m}") for m in range(NMT)]
        with tc.tile_pool(name="wops", bufs=3, space="PSUM") as wops:
            for m in range(NMT):
                for ch in range(4):
                    b_i, h_i = ch // 2, ch % 2
                    sl = (slice(None), b_i, slice(h_i * QH, (h_i + 1) * QH))
                    ps = wops.tile([P, 512], fp32, tag="wo_ps", name="wo_ps")[:, :QH]
                    for ks in range(NMT):
                        nc.tensor.matmul(ps[:], woS[:, ks, m * P:(m + 1) * P],
                                         catT[ks][sl],
                                         start=(ks == 0), stop=False)
                    nc.tensor.matmul(ps[:], ident[:], xT[m][sl],
                                     start=False, stop=True)
                    nc.vector.tensor_scalar_add(x1T[m][sl], ps[:], bo2S[:, m:m + 1])

        # ---- FFN ----------------------------------------------------------
        with tc.tile_pool(name="f1ps", bufs=1, space="PSUM") as f1ps, \
             tc.tile_pool(name="f2ps", bufs=3, space="PSUM") as f2ps, \
             tc.tile_pool(name="outp", bufs=2) as outp:
            ffT = [ffp.tile([P, BPC, N], bfl, tag=f"ffT{m}", name=f"ffT{m}") for m in range(NFT)]
            for mf in range(NFT):
                ps = f1ps.tile([P, 4, 512], fp32, tag="f1", name="f1")
                for ch in range(4):
                    b_i, h_i = ch // 2, ch % 2
                    sl = (slice(None), b_i, slice(h_i * QH, (h_i + 1) * QH))
                    for ks in range(NMT):
                        nc.tensor.matmul(ps[:, ch, :QH],
                                         w1S[:, ks, mf * P:(mf + 1) * P],
                                         x1T[ks][sl],
                                         start=(ks == 0), stop=(ks == NMT - 1))
                nc.scalar.activation(
                    ffT[mf][:].rearrange("p b (h q) -> p (b h) q", q=QH),
                    ps[:, :, :QH], AF.Gelu, bias=b1S[:, mf:mf + 1])
            for m in range(NMT):
                o2 = outp.tile([P, BPC, N], fp32, tag="o2", name="o2")
                for ch in range(4):
                    b_i, h_i = ch // 2, ch % 2
                    sl = (slice(None), b_i, slice(h_i * QH, (h_i + 1) * QH))
                    ps = f2ps.tile([P, 512], fp32, tag="f2", name="f2")[:, :QH]
                    for ks in range(NFT):
                        nc.tensor.matmul(ps[:], w2S[:, ks, m * P:(m + 1) * P],
                                         ffT[ks][sl],
                                         start=(ks == 0), stop=(ks == NFT - 1))
                    nc.vector.scalar_tensor_tensor(o2[sl], ps[:], b2S[:, m:m + 1],
                                                   x1T[m][sl], ALU.add, ALU.add)
                nc.sync.dma_start(out_t[m], o2[:])

        ffp.release()
        act.release()
        const.release()

    nc.compile()
    return nc


_NC_CACHE = None


def kernel(**inputs) -> np.ndarray:
    global _NC_CACHE
    x = np.asarray(inputs["x"], np.float32)
    ln_w = np.asarray(inputs["ln_w"], np.float32)
    ln_b = np.asarray(inputs["ln_b"], np.float32)
    Wq = np.asarray(inputs["Wq"], np.float32)
    Wk = np.asarray(inputs["Wk"], np.float32)
    Wv = np.asarray(inputs["Wv"], np.float32)
    Wo = np.asarray(inputs["Wo"], np.float32)
    bq = np.asarray(inputs["bq"], np.float32)
    bk = np.asarray(inputs["bk"], np.float32)
    bv = np.asarray(inputs["bv"], np.float32)
    bo = np.asarray(inputs["bo"], np.float32)
    rel_bias = np.asarray(inputs["rel_bias"], np.float32)
    W1 = np.asarray(inputs["W1"], np.float32)
    b1 = np.asarray(inputs["b1"], np.float32)
    W2 = np.asarray(inputs["W2"], np.float32)
    b2 = np.asarray(inputs["b2"], np.float32)

    # fold LayerNorm affine into the projections (exact):
    #   xn = xhat * ln_w + ln_b  =>  xn @ W + b = xhat @ (ln_w[:,None]*W) + (ln_b@W + b)
    Wq_f = ln_w[:, None] * Wq
    Wk_f = ln_w[:, None] * Wk
    Wv_f = ln_w[:, None] * Wv
    bq_f = ln_b @ Wq + bq
    bk_f = ln_b @ Wk + bk
    bv_f = ln_b @ Wv + bv
    # v bias passes through softmax-weighted average untouched -> fold into bo
    bo_f = bo + bv_f @ Wo

    master = _build_master(rel_bias)

    nc = _NC_CACHE
    if nc is None:
        nc = _build_nc()
        _NC_CACHE = nc

    shared = {
        "wq": np.ascontiguousarray(Wq_f.astype(bf16)),
        "wk": np.ascontiguousarray(Wk_f.astype(bf16)),
        "wv": np.ascontiguousarray(Wv_f.astype(bf16)),
        "wo": np.ascontiguousarray(Wo.astype(bf16)),
        "bq": bq_f, "bk": bk_f, "bo2": bo_f,
        "w1": np.ascontiguousarray(W1.astype(bf16)), "b1": b1,
        "w2": np.ascontiguousarray(W2.astype(bf16)), "b2": b2,
        "expe": master,
    }
    xr = x.reshape(B, C, N)
    in_maps = []
    for c in range(NCORES):
        m = dict(shared)
        m["xin"] = np.ascontiguousarray(xr[c * BPC:(c + 1) * BPC].astype(bf16))
        in_maps.append(m)

    from concourse.bass_utils import run_bass_kernel_spmd

    res = run_bass_kernel_spmd(
        nc, in_maps, core_ids=list(range(NCORES)),
        trace=bool(int(os.environ.get("KERNEL_TRACE", "0"))),
        tmpdir=os.environ.get("KERNEL_TRACE_DIR") or None,
    )
    if res.exec_time_ns is not None:
        print(f"HW exec time: {res.exec_time_ns} ns", file=sys.stderr)
    outs = [r["out"].reshape(BPC, C, H, W) for r in res.results]
    return np.concatenate(outs, axis=0).astype(np.float32)


if __name__ == "__main__":
    # smoke build
    _build_nc()
    print("build ok")



# revision 14
# speedup vs baseline: 1.0280x; 1.0280x over previous
"""CoAtNet transformer block on 8 trn2 NeuronCores, data-parallel over batch.

Layout strategy (per core, 2 batch elements):
  - Activations live "transposed": [C on partitions, (b, n) on free], which is
    exactly the DRAM layout of x (b, C, H, W).
  - LayerNorm stats via ones-matmul column sums (contraction over partitions).
  - Attention per (batch, head-group of 4, q-half of 392):
      scores^T [k-tokens part, q free] via 4-way row-packed K=32 matmuls,
      exp on ACT straight out of PSUM, multiplicative Toeplitz bias exp(B)
      applied on DVE from a host-built per-head master strip,
      A@V + denominator via 4-way col-packed matmuls (denominator rows are
      broadcast for free by an M=32 ones lhsT).
  - rel_idx is provably k - q + 812 (Toeplitz), so the (16,784,784) bias gather
    reduces to per-head [128,1552] strips built on the host.
  - All matmuls bf16 (fp32 PSUM accumulate); residuals in bf16; output fp32.
"""

import os
import sys

import numpy as np
import ml_dtypes

sys.path.insert(0, "/opt/trn_rl_repo")

H, W, C, HEADS = 28, 28, 512, 16
N = H * W            # 784
FF = 4 * C           # 2048
DH = C // HEADS      # 32
B = 16
NCORES = 8
BPC = B // NCORES    # 2 batch elements per core
P = 128
NMT = C // P         # 4 M-tiles of channels
NKT = 7              # token tiles (6x128 + 16)
NFT = FF // P        # 16
QH = N // 2          # 392 q-half
QQ = N // 4          # 196 q-quarter (attention PSUM granularity)
MCOLS = 1552         # master strip columns
EPS = 1e-5

bf16 = ml_dtypes.bfloat16


def _tok(kt):
    return P if kt < NKT - 1 else N - (NKT - 1) * P  # 128 or 16


def _build_master(rel_bias: np.ndarray) -> np.ndarray:
    """exp of the Toeplitz bias as per-head master strips.

    biasT[k, q] = rel_bias[h, k - q + 812] for k-tile t, row p (k = 128t + p):
    master[h, p, c] with c = q + 768 - 128 t, i.e. master[h,p,c] =
    exp(rel_bias[h, p - c + 1580]) (out-of-range -> exp(0)=1, only reachable
    from invalid k rows which are never contracted).
    """
    padded = np.zeros((HEADS, 1708), np.float32)
    padded[:, : rel_bias.shape[1]] = rel_bias
    e = np.exp(padded)
    idx = 1580 + np.arange(P)[:, None] - np.arange(MCOLS)[None, :]  # (128,1552)
    return np.ascontiguousarray(e[:, idx]).astype(bf16)  # (16,128,1552)


def _build_nc():
    import concourse.bass as bass  # noqa: F401
    import concourse.mybir as mybir
    import concourse.tile as tile
    from concourse import bacc
    from concourse.masks import make_identity

    fp32 = mybir.dt.float32
    bfl = mybir.dt.bfloat16
    ALU = mybir.AluOpType
    AF = mybir.ActivationFunctionType

    nc = bacc.Bacc("TRN2", target_bir_lowering=False, debug=False)

    xin = nc.dram_tensor("xin", (BPC, C, N), bfl, kind="ExternalInput").ap()
    wq = nc.dram_tensor("wq", (C, C), bfl, kind="ExternalInput").ap()
    wk = nc.dram_tensor("wk", (C, C), bfl, kind="ExternalInput").ap()
    wv = nc.dram_tensor("wv", (C, C), bfl, kind="ExternalInput").ap()
    wo = nc.dram_tensor("wo", (C, C), bfl, kind="ExternalInput").ap()
    bq = nc.dram_tensor("bq", (C,), fp32, kind="ExternalInput").ap()
    bk = nc.dram_tensor("bk", (C,), fp32, kind="ExternalInput").ap()
    bo2 = nc.dram_tensor("bo2", (C,), fp32, kind="ExternalInput").ap()
    w1 = nc.dram_tensor("w1", (C, FF), bfl, kind="ExternalInput").ap()
    b1 = nc.dram_tensor("b1", (FF,), fp32, kind="ExternalInput").ap()
    w2 = nc.dram_tensor("w2", (FF, C), bfl, kind="ExternalInput").ap()
    b2 = nc.dram_tensor("b2", (C,), fp32, kind="ExternalInput").ap()
    expe = nc.dram_tensor("expe", (HEADS, P, MCOLS), bfl, kind="ExternalInput").ap()
    out = nc.dram_tensor("out", (BPC, C, N), fp32, kind="ExternalOutput").ap()

    x_t = xin.rearrange("b (mt p) n -> mt p b n", p=P)
    out_t = out.rearrange("b (mt p) n -> mt p b n", p=P)

    with tile.TileContext(nc) as tc:
        # ---- persistent pools -------------------------------------------
        const = tc.alloc_tile_pool(name="const", bufs=1)
        act = tc.alloc_tile_pool(name="act", bufs=1)

        wqS = const.tile([P, NMT, C], bfl, tag="wqS", name="wqS")
        wkS = const.tile([P, NMT, C], bfl, tag="wkS", name="wkS")
        wvS = const.tile([P, NMT, C], bfl, tag="wvS", name="wvS")
        woS = const.tile([P, NMT, C], bfl, tag="woS", name="woS")
        for w_d, w_s in ((wq, wqS), (wk, wkS), (wv, wvS), (wo, woS)):
            nc.sync.dma_start(w_s[:], w_d.rearrange("(ks p) m -> p ks m", p=P))
        bqS = const.tile([P, NMT], fp32, tag="bqS", name="bqS")
        bkS = const.tile([P, NMT], fp32, tag="bkS", name="bkS")
        bo2S = const.tile([P, NMT], fp32, tag="bo2S", name="bo2S")
        b2S = const.tile([P, NMT], fp32, tag="b2S", name="b2S")
        for b_d, b_s in ((bq, bqS), (bk, bkS), (bo2, bo2S), (b2, b2S)):
            nc.sync.dma_start(b_s[:], b_d.rearrange("(mt p) -> p mt", p=P))
        b1S = const.tile([P, NFT], fp32, tag="b1S", name="b1S")
        nc.sync.dma_start(b1S[:], b1.rearrange("(mt p) -> p mt", p=P))

        ones_bf = const.tile([P, DH], bfl, tag="ones_bf", name="ones_bf")
        nc.any.memset(ones_bf[:], 1.0)
        ident = const.tile([P, P], bfl, tag="ident", name="ident")
        make_identity(nc, ident)

        xT = [act.tile([P, BPC, N], bfl, tag=f"xT{m}", name=f"xT{m}") for m in range(NMT)]
        for m in range(NMT):
            nc.sync.dma_start(xT[m][:], x_t[m])


        # ---- LayerNorm stats --------------------------------------------
        with tc.tile_pool(name="lnp", bufs=1) as lnp, \
             tc.tile_pool(name="lnps", bufs=2, space="PSUM") as lnps:
            xsq = [lnp.tile([P, BPC, N], bfl, tag=f"xsq{m}", name=f"xsq{m}") for m in range(NMT)]
            for m in range(NMT):
                nc.scalar.square(xsq[m][:], xT[m][:])
            must = lnp.tile([1, BPC, N], fp32, tag="must", name="must")
            sqst = lnp.tile([1, BPC, N], fp32, tag="sqst", name="sqst")
            for ch in range(4):
                b_i, h_i = ch // 2, ch % 2
                sl = (slice(None), b_i, slice(h_i * QH, (h_i + 1) * QH))
                sp = lnps.tile([P, 512], fp32)
                for ks in range(NMT):
                    nc.tensor.matmul(sp[0:1, :QH], ones_bf[:, 0:1], xT[ks][sl],
                                     start=(ks == 0), stop=(ks == NMT - 1),
                                     tile_position=(0, 0))
                    nc.tensor.matmul(sp[32:33, :QH], ones_bf[:, 0:1], xsq[ks][sl],
                                     start=(ks == 0), stop=(ks == NMT - 1),
                                     tile_position=(0, 32))
                # scale by 1/C on eviction
                nc.vector.tensor_scalar_mul(must[0:1, b_i, sl[2]], sp[0:1, :QH], 1.0 / C)
                nc.vector.tensor_scalar_mul(sqst[0:1, b_i, sl[2]], sp[32:33, :QH], 1.0 / C)
            mu = must[:]        # [1, BPC, N]
            msq = sqst[:]
            var = lnp.tile([1, BPC, N], fp32, tag="var", name="var")
            tmp1 = lnp.tile([1, BPC, N], fp32, tag="tmp1", name="tmp1")
            nc.vector.tensor_mul(tmp1[:], mu, mu)
            # var = (msq + eps) - mu^2
            nc.vector.scalar_tensor_tensor(var[:], msq, float(EPS), tmp1[:],
                                           ALU.add, ALU.subtract)
            sd = lnp.tile([1, BPC, N], fp32, tag="sd", name="sd")
            nc.scalar.activation(sd[:], var[:], AF.Sqrt)
            rsig = lnp.tile([1, BPC, N], fp32, tag="rsig", name="rsig")
            nc.vector.reciprocal_approx_accurate(rsig[:], sd[:], tmp1[:])
            negmur = lnp.tile([1, BPC, N], fp32, tag="negmur", name="negmur")
            nc.vector.scalar_tensor_tensor(negmur[:], mu, -1.0, rsig[:],
                                           ALU.mult, ALU.mult)
            rsig_bf = lnp.tile([1, BPC, N], bfl, tag="rsig_bf", name="rsig_bf")
            negmur_bf = lnp.tile([1, BPC, N], bfl, tag="negmur_bf", name="negmur_bf")
            nc.vector.tensor_copy(rsig_bf[:], rsig[:])
            nc.vector.tensor_copy(negmur_bf[:], negmur[:])
            rsigB = act.tile([P, BPC, N], bfl, tag="rsigB", name="rsigB")
            negmurB = act.tile([P, BPC, N], bfl, tag="negmurB", name="negmurB")
            nc.gpsimd.partition_broadcast(rsigB[:], rsig_bf[:])
            nc.gpsimd.partition_broadcast(negmurB[:], negmur_bf[:])

            # xn = x * rsig + (-mu * rsig)   (ln_w/ln_b folded into weights)
            xnT = [act.tile([P, BPC, N], bfl, tag=f"xnT{m}", name=f"xnT{m}") for m in range(NMT)]
            for m in range(NMT):
                nc.vector.tensor_mul(xsq[m][:], xT[m][:], rsigB[:])
                nc.vector.tensor_add(xnT[m][:], xsq[m][:], negmurB[:])

        # ---- QKV projections --------------------------------------------
        qT = [act.tile([P, BPC, N], bfl, tag=f"qT{m}", name=f"qT{m}") for m in range(NMT)]
        kT = [act.tile([P, BPC, N], bfl, tag=f"kT{m}", name=f"kT{m}") for m in range(NMT)]
        vS = [act.tile([P, NKT, C], bfl, tag=f"vS{b}", name=f"vS{b}") for b in range(BPC)]
        with tc.tile_pool(name="qkvps", bufs=3, space="PSUM") as qkvps:
            for wS, bS, dstT in ((wqS, bqS, qT), (wkS, bkS, kT)):
                for m in range(NMT):
                    for ch in range(4):
                        b_i, h_i = ch // 2, ch % 2
                        sl = (slice(None), b_i, slice(h_i * QH, (h_i + 1) * QH))
                        ps = qkvps.tile([P, 512], fp32, tag="qkv_ps", name="qkv_ps")[:, :QH]
                        for ks in range(NMT):
                            nc.tensor.matmul(ps[:], wS[:, ks, m * P:(m + 1) * P],
                                             xnT[ks][sl],
                                             start=(ks == 0), stop=(ks == NMT - 1))
                        nc.vector.tensor_scalar_add(dstT[m][sl], ps[:], bS[:, m:m + 1])
            # V in token-partition layout: v[b][tok, kt, c_out]
            for b_i in range(BPC):
                for kt in range(NKT):
                    tok = _tok(kt)
                    ps = qkvps.tile([P, C], fp32, tag="v_ps", name="v_ps")
                    for ks in range(NMT):
                        nc.tensor.matmul(
                            ps[:tok, :],
                            xnT[ks][:, b_i, kt * P:kt * P + tok],
                            wvS[:, ks, :],
                            start=(ks == 0), stop=(ks == NMT - 1))
                    nc.vector.tensor_copy(vS[b_i][:tok, kt, :], ps[:tok, :])

        # ---- attention ---------------------------------------------------
        # Scores PSUM is split into two head-pair tiles of [P, 2, 512]
        # (2 banks each, heads bank-aligned). With scps bufs=2 the next
        # iteration's first head-pair scores overlap the current exp,
        # pipelining Tensor against ACT. PSUM: 2x2 + av 2 + dn 2 = 8 banks.
        catT = [act.tile([P, BPC, N], bfl, tag=f"catT{m}", name=f"catT{m}") for m in range(NMT)]
        with tc.tile_pool(name="expp", bufs=2) as expp, \
             tc.tile_pool(name="pp", bufs=4) as pp, \
             tc.tile_pool(name="rp", bufs=2) as rp, \
             tc.tile_pool(name="scps", bufs=3, space="PSUM") as scps, \
             tc.tile_pool(name="avps", bufs=1, space="PSUM") as avps, \
             tc.tile_pool(name="dnps", bufs=1, space="PSUM") as dnps:
            for g in range(4):  # head group: heads 4g..4g+3
                eG = expp.tile([P, 4, MCOLS], bfl, tag="eG", name="eG")
                nc.sync.dma_start(eG[:], expe[4 * g:4 * g + 4].rearrange("h p c -> p h c"))
                for b_i in range(BPC):
                    for qh in range(2):
                        qsl = slice(qh * QH, (qh + 1) * QH)
                        av = avps.tile([P, 512], fp32, tag="av", name="av")[:, :QH]
                        dn = dnps.tile([P, 512], fp32, tag="dn", name="dn")[:, :QH]
                        for kt in range(NKT):
                            tok = _tok(kt)
                            ksl = slice(kt * P, kt * P + tok)
                            scH = [scps.tile([P, 2, 512], fp32, tag="sc", name="sc")
                                   for _ in range(2)]
                            c0 = 768 - P * kt + qh * QH
                            for j in range(4):
                                nc.tensor.matmul(
                                    scH[j // 2][:tok, j % 2, :QH],
                                    kT[g][32 * j:32 * j + 32, b_i, ksl],
                                    qT[g][32 * j:32 * j + 32, b_i, qsl],
                                    start=True, stop=True,
                                    tile_position=(32 * j, 0))
                            for hp in range(2):
                                pS = pp.tile([P, 2, QH], bfl, tag="pS", name="pS")
                                nc.scalar.activation(pS[:tok], scH[hp][:tok, :, :QH],
                                                     AF.Exp)
                                nc.vector.tensor_mul(
                                    pS[:tok], pS[:tok],
                                    eG[:tok, 2 * hp:2 * hp + 2, c0:c0 + QH])
                                for jj in range(2):
                                    j = 2 * hp + jj
                                    nc.tensor.matmul(
                                        av[32 * j:32 * j + 32, :],
                                        vS[b_i][:tok, kt, 128 * g + 32 * j:128 * g + 32 * j + 32],
                                        pS[:tok, jj, :],
                                        start=(kt == 0), stop=(kt == NKT - 1),
                                        tile_position=(0, 32 * j),
                                        skip_group_check=True)
                                    nc.tensor.matmul(
                                        dn[32 * j:32 * j + 32, :],
                                        ones_bf[:tok, :],
                                        pS[:tok, jj, :],
                                        start=(kt == 0), stop=(kt == NKT - 1),
                                        tile_position=(0, 32 * j),
                                        skip_group_check=True)
                        recipB = rp.tile([P, QH], fp32, tag="recipB", name="recipB")
                        scr = rp.tile([P, QH], fp32, tag="scr", name="scr")
                        nc.vector.reciprocal_approx_accurate(recipB[:], dn[:], scr[:])
                        nc.vector.tensor_mul(catT[g][:, b_i, qsl], av[:], recipB[:])

        # ---- out proj + residual ----------------------------------------
        # FFN weight DMAs issued here so they overlap the out-projection.
        ffp = tc.alloc_tile_pool(name="ffp", bufs=1)
        w1S = ffp.tile([P, NMT, FF], bfl, tag="w1S", name="w1S")
        nc.sync.dma_start(w1S[:], w1.rearrange("(ks p) m -> p ks m", p=P))
        w2S = ffp.tile([P, NFT, C], bfl, tag="w2S", name="w2S")
        nc.sync.dma_start(w2S[:], w2.rearrange("(ks p) m -> p ks m", p=P))

        x1T = [act.tile([P, BPC, N], bfl, tag=f"x1T{m}", name=f"x1T{m}") for m in range(NMT)]
        with tc.tile_pool(name="wops", bufs=3, space="PSUM") as wops:
            for m in range(NMT):
                for ch in range(4):
                    b_i, h_i = ch // 2, ch % 2
                    sl = (slice(None), b_i, slice(h_i * QH, (h_i + 1) * QH))
                    ps = wops.tile([P, 512], fp32, tag="wo_ps", name="wo_ps")[:, :QH]
                    for ks in range(NMT):
                        nc.tensor.matmul(ps[:], woS[:, ks, m * P:(m + 1) * P],
                                         catT[ks][sl],
                                         start=(ks == 0), stop=False)
                    nc.tensor.matmul(ps[:], ident[:], xT[m][sl],
                                     start=False, stop=True)
                    nc.vector.tensor_scalar_add(x1T[m][sl], ps[:], bo2S[:, m:m + 1])

        # ---- FFN ----------------------------------------------------------
        with tc.tile_pool(name="f1ps", bufs=1, space="PSUM") as f1ps, \
             tc.tile_pool(name="f2ps", bufs=3, space="PSUM") as f2ps, \
             tc.tile_pool(name="outp", bufs=2) as outp:
            ffT = [ffp.tile([P, BPC, N], bfl, tag=f"ffT{m}", name=f"ffT{m}") for m in range(NFT)]
            for mf in range(NFT):
                ps = f1ps.tile([P, 4, 512], fp32, tag="f1", name="f1")
                for ch in range(4):
                    b_i, h_i = ch // 2, ch % 2
                    sl = (slice(None), b_i, slice(h_i * QH, (h_i + 1) * QH))
                    for ks in range(NMT):
                        nc.tensor.matmul(ps[:, ch, :QH],
                                         w1S[:, ks, mf * P:(mf + 1) * P],
                                         x1T[ks][sl],
                                         start=(ks == 0), stop=(ks == NMT - 1))
                nc.scalar.activation(
                    ffT[mf][:].rearrange("p b (h q) -> p (b h) q", q=QH),
                    ps[:, :, :QH], AF.Gelu, bias=b1S[:, mf:mf + 1])
            for m in range(NMT):
                o2 = outp.tile([P, BPC, N], fp32, tag="o2", name="o2")
                for ch in range(4):
                    b_i, h_i = ch // 2, ch % 2
                    sl = (slice(None), b_i, slice(h_i * QH, (h_i + 1) * QH))
                    ps = f2ps.tile([P, 512], fp32, tag="f2", name="f2")[:, :QH]
                    for ks in range(NFT):
                        nc.tensor.matmul(ps[:], w2S[:, ks, m * P:(m + 1) * P],
                                         ffT[ks][sl],
                                         start=(ks == 0), stop=(ks == NFT - 1))
                    nc.vector.scalar_tensor_tensor(o2[sl], ps[:], b2S[:, m:m + 1],
                                                   x1T[m][sl], ALU.add, ALU.add)
                nc.sync.dma_start(out_t[m], o2[:])

        ffp.release()
        act.release()
        const.release()

    nc.compile()
    return nc


_NC_CACHE = None


def kernel(**inputs) -> np.ndarray:
    global _NC_CACHE
    x = np.asarray(inputs["x"], np.float32)
    ln_w = np.asarray(inputs["ln_w"], np.float32)
    ln_b = np.asarray(inputs["ln_b"], np.float32)
    Wq = np.asarray(inputs["Wq"], np.float32)
    Wk = np.asarray(inputs["Wk"], np.float32)
    Wv = np.asarray(inputs["Wv"], np.float32)
    Wo = np.asarray(inputs["Wo"], np.float32)
    bq = np.asarray(inputs["bq"], np.float32)
    bk = np.asarray(inputs["bk"], np.float32)
    bv = np.asarray(inputs["bv"], np.float32)
    bo = np.asarray(inputs["bo"], np.float32)
    rel_bias = np.asarray(inputs["rel_bias"], np.float32)
    W1 = np.asarray(inputs["W1"], np.float32)
    b1 = np.asarray(inputs["b1"], np.float32)
    W2 = np.asarray(inputs["W2"], np.float32)
    b2 = np.asarray(inputs["b2"], np.float32)

    # fold LayerNorm affine into the projections (exact):
    #   xn = xhat * ln_w + ln_b  =>  xn @ W + b = xhat @ (ln_w[:,None]*W) + (ln_b@W + b)
    Wq_f = ln_w[:, None] * Wq
    Wk_f = ln_w[:, None] * Wk
    Wv_f = ln_w[:, None] * Wv
    bq_f = ln_b @ Wq + bq
    bk_f = ln_b @ Wk + bk
    bv_f = ln_b @ Wv + bv
    # v bias passes through softmax-weighted average untouched -> fold into bo
    bo_f = bo + bv_f @ Wo

    master = _build_master(rel_bias)

    nc = _NC_CACHE
    if nc is None:
        nc = _build_nc()
        _NC_CACHE = nc

    shared = {
        "wq": np.ascontiguousarray(Wq_f.astype(bf16)),
        "wk": np.ascontiguousarray(Wk_f.astype(bf16)),
        "wv": np.ascontiguousarray(Wv_f.astype(bf16)),
        "wo": np.ascontiguousarray(Wo.astype(bf16)),
        "bq": bq_f, "bk": bk_f, "bo2": bo_f,
        "w1": np.ascontiguousarray(W1.astype(bf16)), "b1": b1,
        "w2": np.ascontiguousarray(W2.astype(bf16)), "b2": b2,
        "expe": master,
    }
    xr = x.reshape(B, C, N)
    in_maps = []
    for c in range(NCORES):
        m = dict(shared)
        m["xin"] = np.ascontiguousarray(xr[c * BPC:(c + 1) * BPC].astype(bf16))
        in_maps.append(m)

    from concourse.bass_utils import run_bass_kernel_spmd

    res = run_bass_kernel_spmd(
        nc, in_maps, core_ids=list(range(NCORES)),
        trace=bool(int(os.environ.get("KERNEL_TRACE", "0"))),
        tmpdir=os.environ.get("KERNEL_TRACE_DIR") or None,
    )
    if res.exec_time_ns is not None:
        print(f"HW exec time: {res.exec_time_ns} ns", file=sys.stderr)
    outs = [r["out"].reshape(BPC, C, H, W) for r in res.results]
    return np.concatenate(outs, axis=0).astype(np.float32)


if __name__ == "__main__":
    # smoke build
    _build_nc()
    print("build ok")



# revision 16
# speedup vs baseline: 1.2008x; 1.1681x over previous
"""CoAtNet transformer block on 8 trn2 NeuronCores, data-parallel over batch.

Layout strategy (per core, 2 batch elements):
  - Activations live "transposed": [C on partitions, (b, n) on free], which is
    exactly the DRAM layout of x (b, C, H, W).
  - LayerNorm stats via ones-matmul column sums (contraction over partitions).
  - Attention per (batch, head-group of 4, q-half of 392):
      scores^T [k-tokens part, q free] via 4-way row-packed K=32 matmuls,
      exp on ACT straight out of PSUM, multiplicative Toeplitz bias exp(B)
      applied on DVE from a host-built per-head master strip,
      A@V + denominator via 4-way col-packed matmuls (denominator rows are
      broadcast for free by an M=32 ones lhsT).
  - rel_idx is provably k - q + 812 (Toeplitz), so the (16,784,784) bias gather
    reduces to per-head [128,1552] strips built on the host.
  - All matmuls bf16 (fp32 PSUM accumulate); residuals in bf16; output fp32.
"""

import os
import sys

import numpy as np
import ml_dtypes

sys.path.insert(0, "/opt/trn_rl_repo")

H, W, C, HEADS = 28, 28, 512, 16
N = H * W            # 784
FF = 4 * C           # 2048
DH = C // HEADS      # 32
B = 16
NCORES = 8
BPC = B // NCORES    # 2 batch elements per core
P = 128
NMT = C // P         # 4 M-tiles of channels
NKT = 7              # token tiles (6x128 + 16)
NFT = FF // P        # 16
QH = N // 2          # 392 q-half
QQ = N // 4          # 196 q-quarter (attention PSUM granularity)
MCOLS = 1552         # master strip columns
EPS = 1e-5

bf16 = ml_dtypes.bfloat16


def _tok(kt):
    return P if kt < NKT - 1 else N - (NKT - 1) * P  # 128 or 16


def _build_master(rel_bias: np.ndarray) -> np.ndarray:
    """exp of the Toeplitz bias as per-head master strips.

    biasT[k, q] = rel_bias[h, k - q + 812] for k-tile t, row p (k = 128t + p):
    master[h, p, c] with c = q + 768 - 128 t, i.e. master[h,p,c] =
    exp(rel_bias[h, p - c + 1580]) (out-of-range -> exp(0)=1, only reachable
    from invalid k rows which are never contracted).
    """
    padded = np.zeros((HEADS, 1708), np.float32)
    padded[:, : rel_bias.shape[1]] = rel_bias
    e = np.exp(padded)
    idx = 1580 + np.arange(P)[:, None] - np.arange(MCOLS)[None, :]  # (128,1552)
    return np.ascontiguousarray(e[:, idx]).astype(bf16)  # (16,128,1552)


def _build_nc():
    import concourse.bass as bass  # noqa: F401
    import concourse.mybir as mybir
    import concourse.tile as tile
    from concourse import bacc
    from concourse.masks import make_identity

    fp32 = mybir.dt.float32
    bfl = mybir.dt.bfloat16
    ALU = mybir.AluOpType
    AF = mybir.ActivationFunctionType

    nc = bacc.Bacc("TRN2", target_bir_lowering=False, debug=False)

    xin = nc.dram_tensor("xin", (BPC, C, N), bfl, kind="ExternalInput").ap()
    wq = nc.dram_tensor("wq", (C, C), bfl, kind="ExternalInput").ap()
    wk = nc.dram_tensor("wk", (C, C), bfl, kind="ExternalInput").ap()
    wv = nc.dram_tensor("wv", (C, C), bfl, kind="ExternalInput").ap()
    wo = nc.dram_tensor("wo", (C, C), bfl, kind="ExternalInput").ap()
    bq = nc.dram_tensor("bq", (C,), fp32, kind="ExternalInput").ap()
    bk = nc.dram_tensor("bk", (C,), fp32, kind="ExternalInput").ap()
    bo2 = nc.dram_tensor("bo2", (C,), fp32, kind="ExternalInput").ap()
    w1 = nc.dram_tensor("w1", (C, FF), bfl, kind="ExternalInput").ap()
    b1 = nc.dram_tensor("b1", (FF,), fp32, kind="ExternalInput").ap()
    w2 = nc.dram_tensor("w2", (FF, C), bfl, kind="ExternalInput").ap()
    b2 = nc.dram_tensor("b2", (C,), fp32, kind="ExternalInput").ap()
    expe = nc.dram_tensor("expe", (HEADS, P, MCOLS), bfl, kind="ExternalInput").ap()
    out = nc.dram_tensor("out", (BPC, C, N), fp32, kind="ExternalOutput").ap()

    x_t = xin.rearrange("b (mt p) n -> mt p b n", p=P)
    out_t = out.rearrange("b (mt p) n -> mt p b n", p=P)

    with tile.TileContext(nc) as tc:
        # ---- persistent pools -------------------------------------------
        const = tc.alloc_tile_pool(name="const", bufs=1)
        act = tc.alloc_tile_pool(name="act", bufs=1)

        wqS = const.tile([P, NMT, C], bfl, tag="wqS", name="wqS")
        wkS = const.tile([P, NMT, C], bfl, tag="wkS", name="wkS")
        wvS = const.tile([P, NMT, C], bfl, tag="wvS", name="wvS")
        woS = const.tile([P, NMT, C], bfl, tag="woS", name="woS")
        for w_d, w_s in ((wq, wqS), (wk, wkS), (wv, wvS), (wo, woS)):
            nc.sync.dma_start(w_s[:], w_d.rearrange("(ks p) m -> p ks m", p=P))
        bqS = const.tile([P, NMT], fp32, tag="bqS", name="bqS")
        bkS = const.tile([P, NMT], fp32, tag="bkS", name="bkS")
        bo2S = const.tile([P, NMT], fp32, tag="bo2S", name="bo2S")
        b2S = const.tile([P, NMT], fp32, tag="b2S", name="b2S")
        for b_d, b_s in ((bq, bqS), (bk, bkS), (bo2, bo2S), (b2, b2S)):
            nc.sync.dma_start(b_s[:], b_d.rearrange("(mt p) -> p mt", p=P))
        b1S = const.tile([P, NFT], fp32, tag="b1S", name="b1S")
        nc.sync.dma_start(b1S[:], b1.rearrange("(mt p) -> p mt", p=P))

        ones_bf = const.tile([P, DH], bfl, tag="ones_bf", name="ones_bf")
        nc.any.memset(ones_bf[:], 1.0)
        ident = const.tile([P, P], bfl, tag="ident", name="ident")
        make_identity(nc, ident)

        xT = [act.tile([P, BPC, N], bfl, tag=f"xT{m}", name=f"xT{m}") for m in range(NMT)]
        for m in range(NMT):
            nc.sync.dma_start(xT[m][:], x_t[m])


        # ---- LayerNorm stats --------------------------------------------
        with tc.tile_pool(name="lnp", bufs=1) as lnp, \
             tc.tile_pool(name="lnps", bufs=2, space="PSUM") as lnps:
            xsq = [lnp.tile([P, BPC, N], bfl, tag=f"xsq{m}", name=f"xsq{m}") for m in range(NMT)]
            for m in range(NMT):
                nc.scalar.square(xsq[m][:], xT[m][:])
            must = lnp.tile([1, BPC, N], fp32, tag="must", name="must")
            sqst = lnp.tile([1, BPC, N], fp32, tag="sqst", name="sqst")
            for ch in range(4):
                b_i, h_i = ch // 2, ch % 2
                sl = (slice(None), b_i, slice(h_i * QH, (h_i + 1) * QH))
                sp = lnps.tile([P, 512], fp32)
                for ks in range(NMT):
                    nc.tensor.matmul(sp[0:1, :QH], ones_bf[:, 0:1], xT[ks][sl],
                                     start=(ks == 0), stop=(ks == NMT - 1),
                                     tile_position=(0, 0))
                    nc.tensor.matmul(sp[32:33, :QH], ones_bf[:, 0:1], xsq[ks][sl],
                                     start=(ks == 0), stop=(ks == NMT - 1),
                                     tile_position=(0, 32))
                # scale by 1/C on eviction
                nc.vector.tensor_scalar_mul(must[0:1, b_i, sl[2]], sp[0:1, :QH], 1.0 / C)
                nc.vector.tensor_scalar_mul(sqst[0:1, b_i, sl[2]], sp[32:33, :QH], 1.0 / C)
            mu = must[:]        # [1, BPC, N]
            msq = sqst[:]
            var = lnp.tile([1, BPC, N], fp32, tag="var", name="var")
            tmp1 = lnp.tile([1, BPC, N], fp32, tag="tmp1", name="tmp1")
            nc.vector.tensor_mul(tmp1[:], mu, mu)
            # var = (msq + eps) - mu^2
            nc.vector.scalar_tensor_tensor(var[:], msq, float(EPS), tmp1[:],
                                           ALU.add, ALU.subtract)
            sd = lnp.tile([1, BPC, N], fp32, tag="sd", name="sd")
            nc.scalar.activation(sd[:], var[:], AF.Sqrt)
            rsig = lnp.tile([1, BPC, N], fp32, tag="rsig", name="rsig")
            nc.vector.reciprocal_approx_accurate(rsig[:], sd[:], tmp1[:])
            negmur = lnp.tile([1, BPC, N], fp32, tag="negmur", name="negmur")
            nc.vector.scalar_tensor_tensor(negmur[:], mu, -1.0, rsig[:],
                                           ALU.mult, ALU.mult)
            rsig_bf = lnp.tile([1, BPC, N], bfl, tag="rsig_bf", name="rsig_bf")
            negmur_bf = lnp.tile([1, BPC, N], bfl, tag="negmur_bf", name="negmur_bf")
            nc.vector.tensor_copy(rsig_bf[:], rsig[:])
            nc.vector.tensor_copy(negmur_bf[:], negmur[:])
            rsigB = act.tile([P, BPC, N], bfl, tag="rsigB", name="rsigB")
            negmurB = act.tile([P, BPC, N], bfl, tag="negmurB", name="negmurB")
            nc.gpsimd.partition_broadcast(rsigB[:], rsig_bf[:])
            nc.gpsimd.partition_broadcast(negmurB[:], negmur_bf[:])

            # xn = x * rsig + (-mu * rsig)   (ln_w/ln_b folded into weights)
            xnT = [act.tile([P, BPC, N], bfl, tag=f"xnT{m}", name=f"xnT{m}") for m in range(NMT)]
            for m in range(NMT):
                nc.vector.tensor_mul(xsq[m][:], xT[m][:], rsigB[:])
                nc.vector.tensor_add(xnT[m][:], xsq[m][:], negmurB[:])

        # ---- QKV projections --------------------------------------------
        qT = [act.tile([P, BPC, N], bfl, tag=f"qT{m}", name=f"qT{m}") for m in range(NMT)]
        kT = [act.tile([P, BPC, N], bfl, tag=f"kT{m}", name=f"kT{m}") for m in range(NMT)]
        vS = [act.tile([P, NKT, C], bfl, tag=f"vS{b}", name=f"vS{b}") for b in range(BPC)]
        with tc.tile_pool(name="qkvps", bufs=3, space="PSUM") as qkvps:
            for wS, bS, dstT in ((wqS, bqS, qT), (wkS, bkS, kT)):
                for m in range(NMT):
                    for ch in range(4):
                        b_i, h_i = ch // 2, ch % 2
                        sl = (slice(None), b_i, slice(h_i * QH, (h_i + 1) * QH))
                        ps = qkvps.tile([P, 512], fp32, tag="qkv_ps", name="qkv_ps")[:, :QH]
                        for ks in range(NMT):
                            nc.tensor.matmul(ps[:], wS[:, ks, m * P:(m + 1) * P],
                                             xnT[ks][sl],
                                             start=(ks == 0), stop=(ks == NMT - 1))
                        nc.vector.tensor_scalar_add(dstT[m][sl], ps[:], bS[:, m:m + 1])
            # V in token-partition layout: v[b][tok, kt, c_out]
            for b_i in range(BPC):
                for kt in range(NKT):
                    tok = _tok(kt)
                    ps = qkvps.tile([P, C], fp32, tag="v_ps", name="v_ps")
                    for ks in range(NMT):
                        nc.tensor.matmul(
                            ps[:tok, :],
                            xnT[ks][:, b_i, kt * P:kt * P + tok],
                            wvS[:, ks, :],
                            start=(ks == 0), stop=(ks == NMT - 1))
                    nc.vector.tensor_copy(vS[b_i][:tok, kt, :], ps[:tok, :])

        # ---- attention ---------------------------------------------------
        # Scores PSUM is split into two head-pair tiles of [P, 2, 512]
        # (2 banks each, heads bank-aligned). With scps bufs=2 the next
        # iteration's first head-pair scores overlap the current exp,
        # pipelining Tensor against ACT. PSUM: 2x2 + av 2 + dn 2 = 8 banks.
        catT = [act.tile([P, BPC, N], bfl, tag=f"catT{m}", name=f"catT{m}") for m in range(NMT)]
        with tc.tile_pool(name="expp", bufs=2) as expp, \
             tc.tile_pool(name="pp", bufs=6) as pp, \
             tc.tile_pool(name="rp", bufs=2) as rp, \
             tc.tile_pool(name="scps", bufs=3, space="PSUM") as scps, \
             tc.tile_pool(name="avps", bufs=1, space="PSUM") as avps, \
             tc.tile_pool(name="dnps", bufs=1, space="PSUM") as dnps:
            for g in range(4):  # head group: heads 4g..4g+3
                eG = expp.tile([P, 4, MCOLS], bfl, tag="eG", name="eG")
                nc.sync.dma_start(eG[:], expe[4 * g:4 * g + 4].rearrange("h p c -> p h c"))
                for b_i in range(BPC):
                    for qh in range(2):
                        qsl = slice(qh * QH, (qh + 1) * QH)
                        av = avps.tile([P, 512], fp32, tag="av", name="av")[:, :QH]
                        dn = dnps.tile([P, 512], fp32, tag="dn", name="dn")[:, :QH]

                        def av_dn(kt, pS_pair):
                            # A@V + denominator for one kt tile, emitted one
                            # iteration late so ready scores sit ahead of
                            # these (possibly blocked) matmuls in the queue.
                            tok = _tok(kt)
                            for hp in range(2):
                                for jj in range(2):
                                    j = 2 * hp + jj
                                    nc.tensor.matmul(
                                        av[32 * j:32 * j + 32, :],
                                        vS[b_i][:tok, kt, 128 * g + 32 * j:128 * g + 32 * j + 32],
                                        pS_pair[hp][:tok, jj, :],
                                        start=(kt == 0), stop=(kt == NKT - 1),
                                        tile_position=(0, 32 * j),
                                        skip_group_check=True)
                                    nc.tensor.matmul(
                                        dn[32 * j:32 * j + 32, :],
                                        ones_bf[:tok, :],
                                        pS_pair[hp][:tok, jj, :],
                                        start=(kt == 0), stop=(kt == NKT - 1),
                                        tile_position=(0, 32 * j),
                                        skip_group_check=True)

                        prev = None
                        for kt in range(NKT):
                            tok = _tok(kt)
                            ksl = slice(kt * P, kt * P + tok)
                            scH = [scps.tile([P, 2, 512], fp32, tag="sc", name="sc")
                                   for _ in range(2)]
                            c0 = 768 - P * kt + qh * QH
                            for j in range(4):
                                nc.tensor.matmul(
                                    scH[j // 2][:tok, j % 2, :QH],
                                    kT[g][32 * j:32 * j + 32, b_i, ksl],
                                    qT[g][32 * j:32 * j + 32, b_i, qsl],
                                    start=True, stop=True,
                                    tile_position=(32 * j, 0))
                            pS_pair = []
                            for hp in range(2):
                                pS = pp.tile([P, 2, QH], bfl, tag="pS", name="pS")
                                nc.scalar.activation(pS[:tok], scH[hp][:tok, :, :QH],
                                                     AF.Exp)
                                nc.vector.tensor_mul(
                                    pS[:tok], pS[:tok],
                                    eG[:tok, 2 * hp:2 * hp + 2, c0:c0 + QH])
                                pS_pair.append(pS)
                            if prev is not None:
                                av_dn(*prev)
                            prev = (kt, pS_pair)
                        av_dn(*prev)
                        recipB = rp.tile([P, QH], fp32, tag="recipB", name="recipB")
                        scr = rp.tile([P, QH], fp32, tag="scr", name="scr")
                        nc.vector.reciprocal_approx_accurate(recipB[:], dn[:], scr[:])
                        nc.vector.tensor_mul(catT[g][:, b_i, qsl], av[:], recipB[:])

        # ---- out proj + residual ----------------------------------------
        # FFN weight DMAs issued here so they overlap the out-projection.
        ffp = tc.alloc_tile_pool(name="ffp", bufs=1)
        w1S = ffp.tile([P, NMT, FF], bfl, tag="w1S", name="w1S")
        nc.sync.dma_start(w1S[:], w1.rearrange("(ks p) m -> p ks m", p=P))
        w2S = ffp.tile([P, NFT, C], bfl, tag="w2S", name="w2S")
        nc.sync.dma_start(w2S[:], w2.rearrange("(ks p) m -> p ks m", p=P))

        x1T = [act.tile([P, BPC, N], bfl, tag=f"x1T{m}", name=f"x1T{m}") for m in range(NMT)]
        with tc.tile_pool(name="wops", bufs=3, space="PSUM") as wops:
            for m in range(NMT):
                for ch in range(4):
                    b_i, h_i = ch // 2, ch % 2
                    sl = (slice(None), b_i, slice(h_i * QH, (h_i + 1) * QH))
                    ps = wops.tile([P, 512], fp32, tag="wo_ps", name="wo_ps")[:, :QH]
                    for ks in range(NMT):
                        nc.tensor.matmul(ps[:], woS[:, ks, m * P:(m + 1) * P],
                                         catT[ks][sl],
                                         start=(ks == 0), stop=False)
                    nc.tensor.matmul(ps[:], ident[:], xT[m][sl],
                                     start=False, stop=True)
                    nc.vector.tensor_scalar_add(x1T[m][sl], ps[:], bo2S[:, m:m + 1])

        # ---- FFN ----------------------------------------------------------
        with tc.tile_pool(name="f1ps", bufs=1, space="PSUM") as f1ps, \
             tc.tile_pool(name="f2ps", bufs=3, space="PSUM") as f2ps, \
             tc.tile_pool(name="outp", bufs=2) as outp:
            ffT = [ffp.tile([P, BPC, N], bfl, tag=f"ffT{m}", name=f"ffT{m}") for m in range(NFT)]
            for mf in range(NFT):
                ps = f1ps.tile([P, 4, 512], fp32, tag="f1", name="f1")
                for ch in range(4):
                    b_i, h_i = ch // 2, ch % 2
                    sl = (slice(None), b_i, slice(h_i * QH, (h_i + 1) * QH))
                    for ks in range(NMT):
                        nc.tensor.matmul(ps[:, ch, :QH],
                                         w1S[:, ks, mf * P:(mf + 1) * P],
                                         x1T[ks][sl],
                                         start=(ks == 0), stop=(ks == NMT - 1))
                nc.scalar.activation(
                    ffT[mf][:].rearrange("p b (h q) -> p (b h) q", q=QH),
                    ps[:, :, :QH], AF.Gelu, bias=b1S[:, mf:mf + 1])
            for m in range(NMT):
                o2 = outp.tile([P, BPC, N], fp32, tag="o2", name="o2")
                for ch in range(4):
                    b_i, h_i = ch // 2, ch % 2
                    sl = (slice(None), b_i, slice(h_i * QH, (h_i + 1) * QH))
                    ps = f2ps.tile([P, 512], fp32, tag="f2", name="f2")[:, :QH]
                    for ks in range(NFT):
                        nc.tensor.matmul(ps[:], w2S[:, ks, m * P:(m + 1) * P],
                                         ffT[ks][sl],
                                         start=(ks == 0), stop=(ks == NFT - 1))
                    nc.vector.scalar_tensor_tensor(o2[sl], ps[:], b2S[:, m:m + 1],
                                                   x1T[m][sl], ALU.add, ALU.add)
                nc.sync.dma_start(out_t[m], o2[:])

        ffp.release()
        act.release()
        const.release()

    nc.compile()
    return nc


_NC_CACHE = None


def kernel(**inputs) -> np.ndarray:
    global _NC_CACHE
    x = np.asarray(inputs["x"], np.float32)
    ln_w = np.asarray(inputs["ln_w"], np.float32)
    ln_b = np.asarray(inputs["ln_b"], np.float32)
    Wq = np.asarray(inputs["Wq"], np.float32)
    Wk = np.asarray(inputs["Wk"], np.float32)
    Wv = np.asarray(inputs["Wv"], np.float32)
    Wo = np.asarray(inputs["Wo"], np.float32)
    bq = np.asarray(inputs["bq"], np.float32)
    bk = np.asarray(inputs["bk"], np.float32)
    bv = np.asarray(inputs["bv"], np.float32)
    bo = np.asarray(inputs["bo"], np.float32)
    rel_bias = np.asarray(inputs["rel_bias"], np.float32)
    W1 = np.asarray(inputs["W1"], np.float32)
    b1 = np.asarray(inputs["b1"], np.float32)
    W2 = np.asarray(inputs["W2"], np.float32)
    b2 = np.asarray(inputs["b2"], np.float32)

    # fold LayerNorm affine into the projections (exact):
    #   xn = xhat * ln_w + ln_b  =>  xn @ W + b = xhat @ (ln_w[:,None]*W) + (ln_b@W + b)
    Wq_f = ln_w[:, None] * Wq
    Wk_f = ln_w[:, None] * Wk
    Wv_f = ln_w[:, None] * Wv
    bq_f = ln_b @ Wq + bq
    bk_f = ln_b @ Wk + bk
    bv_f = ln_b @ Wv + bv
    # v bias passes through softmax-weighted average untouched -> fold into bo
    bo_f = bo + bv_f @ Wo

    master = _build_master(rel_bias)

    nc = _NC_CACHE
    if nc is None:
        nc = _build_nc()
        _NC_CACHE = nc

    shared = {
        "wq": np.ascontiguousarray(Wq_f.astype(bf16)),
        "wk": np.ascontiguousarray(Wk_f.astype(bf16)),
        "wv": np.ascontiguousarray(Wv_f.astype(bf16)),
        "wo": np.ascontiguousarray(Wo.astype(bf16)),
        "bq": bq_f, "bk": bk_f, "bo2": bo_f,
        "w1": np.ascontiguousarray(W1.astype(bf16)), "b1": b1,
        "w2": np.ascontiguousarray(W2.astype(bf16)), "b2": b2,
        "expe": master,
    }
    xr = x.reshape(B, C, N)
    in_maps = []
    for c in range(NCORES):
        m = dict(shared)
        m["xin"] = np.ascontiguousarray(xr[c * BPC:(c + 1) * BPC].astype(bf16))
        in_maps.append(m)

    from concourse.bass_utils import run_bass_kernel_spmd

    res = run_bass_kernel_spmd(
        nc, in_maps, core_ids=list(range(NCORES)),
        trace=bool(int(os.environ.get("KERNEL_TRACE", "0"))),
        tmpdir=os.environ.get("KERNEL_TRACE_DIR") or None,
    )
    if res.exec_time_ns is not None:
        print(f"HW exec time: {res.exec_time_ns} ns", file=sys.stderr)
    outs = [r["out"].reshape(BPC, C, H, W) for r in res.results]
    return np.concatenate(outs, axis=0).astype(np.float32)


if __name__ == "__main__":
    # smoke build
    _build_nc()
    print("build ok")

